# revision 9
# baseline (speedup 1.0000x reference)
"""Trainium2 Bass kernel for nn_Net_20916490732073 (gnn_message_passing).

Net: LSTMCell(1024->256) on B=16384 states -> 2-layer MLP -> gather to
A=327680 actions (actionToStateMapping) -> 3-layer action MLP -> scalar/action.

Sharding: data-parallel over B across 8 NeuronCores. Each action row is
routed (on host) to the core that owns its state, so the gather is core-local.
On-chip the gather is a SWDGE dma_gather(transpose=True) from an SBUF
token-major z2 buffer, landing feature-major [128, n_actions] tiles that feed
the action-MLP matmuls directly.

All matmuls run in fp16 (fp32 matmul is 4x slower on the PE); elementwise math
and outputs stay fp32.
"""

import numpy as np

H = 256
S = 1024
ASZ = 113  # ACTION_STATE_SIZE
B = 16384
A = 327680
NCORES = 8
BL = B // NCORES  # 2048 states per core
NJ = BL // 512  # 4 batch chunks of 512
CG = 2048  # actions per group (1 gather + 4 MLP chunks of 512)

_PROGRAM_CACHE = {}


def _host_prep(x, lstmState, perActionStates, actionToStateMapping,
               W_ih, W_hh, b_ih, b_hh, W1, b1, W2, b2,
               Wa1, ba1, Wa2, ba2, Wa3, ba3):
    """Partition + transpose inputs per core. Returns (in_maps, NA, perms)."""
    f16 = np.float16
    f32 = np.float32
    core_of = (actionToStateMapping // BL).astype(np.int64)
    perms = [np.nonzero(core_of == d)[0] for d in range(NCORES)]
    maxcnt = max(int(p.size) for p in perms)
    NA = ((maxcnt + CG - 1) // CG) * CG

    # replicated weights (transposed for the PE's lhsT convention)
    wihT = np.ascontiguousarray(W_ih.T.astype(f16))            # [1024, 1024]
    whhT = np.ascontiguousarray(W_hh.T.astype(f16))            # [256, 1024]
    w1T = np.ascontiguousarray(W1.T.astype(f16))               # [256, 128]
    w2T = np.ascontiguousarray(W2.T.astype(f16))               # [128, 128]
    wa1sT = np.ascontiguousarray(Wa1[:, :ASZ].T.astype(f16))   # [113, 64]
    wa1zT = np.ascontiguousarray(Wa1[:, ASZ:].T.astype(f16))   # [128, 64]
# M padded to 32 with zero cols so the packed matmuls initialize the
    # full psum partition range (matmul time scales with N, not M)
    wa2x2 = np.zeros((128, 32), f16)                           # Wa2.T stacked twice
    wa2x2[0:64, 0:4] = Wa2.T
    wa2x2[64:128, 0:4] = Wa2.T
    wa3x4 = np.zeros((128, 32), f16)                           # Wa3.T at rows 32r
    for r in range(4):
        wa3x4[32 * r:32 * r + 4, 0] = Wa3[0, :]
    bsum = (b_ih + b_hh).astype(f32).reshape(8, 128).T         # [128, 8] col m = chunk m
    bsum = np.ascontiguousarray(bsum)
    b1c = np.ascontiguousarray(b1.astype(f32).reshape(128, 1))
    b2bc = np.ascontiguousarray(np.tile(b2.astype(f32)[None, :], (128, 1)))  # [128,128]
    ba1x2 = np.zeros((128, 1), f32)
    ba1x2[0:64, 0] = ba1
    ba1x2[64:128, 0] = ba1
    ba2x4 = np.zeros((128, 1), f32)
    ba3x4 = np.zeros((128, 1), f32)
    for r in range(4):
        ba2x4[32 * r:32 * r + 4, 0] = ba2
        ba3x4[32 * r, 0] = ba3[0]

    shared = dict(wih=wihT, whh=whhT, w1=w1T, w2=w2T, wa1s=wa1sT, wa1z=wa1zT,
                  wa2=wa2x2, wa3=wa3x4, bsum=bsum, b1=b1c, b2bc=b2bc,
                  ba1=ba1x2, ba2=ba2x4, ba3=ba3x4)

    in_maps = []
    for d in range(NCORES):
        sl = slice(d * BL, (d + 1) * BL)
        xT = np.ascontiguousarray(x[sl].T.astype(f16))                 # [1024, 2048]
        h0T = np.ascontiguousarray(lstmState[sl, :H].T.astype(f16))    # [256, 2048]
        c0T = np.ascontiguousarray(lstmState[sl, H:].T.astype(f32))    # [256, 2048]
        perm = perms[d]
        cnt = perm.size
        pasT = np.zeros((ASZ, NA), f16)
        pasT[:, :cnt] = perActionStates[perm].T
        local = np.zeros(NA, np.int16)
        local[:cnt] = (actionToStateMapping[perm] - d * BL).astype(np.int16)
        # dma_gather index wrap: idx i lives at [i % 16, i // 16], replicated
        # to 128 partitions (8 Q7 cores x 16).
        mp16 = local.reshape(NA // 16, 16).T                           # [16, NA/16]
        mp = np.ascontiguousarray(np.tile(mp16, (8, 1)))               # [128, NA/16]
        in_maps.append(dict(xT=xT, h0T=h0T, c0T=c0T, pasT=pasT, mp=mp, **shared))
    return in_maps, NA, perms


def _build_program(NA, mode="full"):
    from concourse import bacc
    import concourse.tile as tile
    import concourse.mybir as mybir

    f32 = mybir.dt.float32
    f16 = mybir.dt.float16
    i16 = mybir.dt.int16
    FT = mybir.ActivationFunctionType
    OP = mybir.AluOpType
    NG = NA // CG  # action groups

    nc = bacc.Bacc("TRN2", num_devices=NCORES, debug=False,
                   target_bir_lowering=False)

    # ---- DRAM tensors -------------------------------------------------
    xT_d = nc.dram_tensor("xT", [S, BL], f16, kind="ExternalInput")
    h0T_d = nc.dram_tensor("h0T", [H, BL], f16, kind="ExternalInput")
    c0T_d = nc.dram_tensor("c0T", [H, BL], f32, kind="ExternalInput")
    pasT_d = nc.dram_tensor("pasT", [ASZ, NA], f16, kind="ExternalInput")
    mp_d = nc.dram_tensor("mp", [128, NA // 16], i16, kind="ExternalInput")
    wih_d = nc.dram_tensor("wih", [S, 4 * H], f16, kind="ExternalInput")
    whh_d = nc.dram_tensor("whh", [H, 4 * H], f16, kind="ExternalInput")
    w1_d = nc.dram_tensor("w1", [H, 128], f16, kind="ExternalInput")
    w2_d = nc.dram_tensor("w2", [128, 128], f16, kind="ExternalInput")
    wa1s_d = nc.dram_tensor("wa1s", [ASZ, 64], f16, kind="ExternalInput")
    wa1z_d = nc.dram_tensor("wa1z", [128, 64], f16, kind="ExternalInput")
    wa2_d = nc.dram_tensor("wa2", [128, 32], f16, kind="ExternalInput")
    wa3_d = nc.dram_tensor("wa3", [128, 32], f16, kind="ExternalInput")
    bsum_d = nc.dram_tensor("bsum", [128, 8], f32, kind="ExternalInput")
    b1_d = nc.dram_tensor("b1", [128, 1], f32, kind="ExternalInput")
    b2bc_d = nc.dram_tensor("b2bc", [128, 128], f32, kind="ExternalInput")
    ba1_d = nc.dram_tensor("ba1", [128, 1], f32, kind="ExternalInput")
    ba2_d = nc.dram_tensor("ba2", [128, 1], f32, kind="ExternalInput")
    ba3_d = nc.dram_tensor("ba3", [128, 1], f32, kind="ExternalInput")
    hcT_d = nc.dram_tensor("hcT", [2 * H, BL], f32, kind="ExternalOutput")
    av4_d = nc.dram_tensor("av4", [4, NA // 4], f32, kind="ExternalOutput")

    from contextlib import ExitStack

    with ExitStack() as ctx:
        tc = ctx.enter_context(tile.TileContext(nc))
        cp = ctx.enter_context(tc.tile_pool(name="consts", bufs=1))
        xp = ctx.enter_context(tc.tile_pool(name="xin", bufs=2))
        hp = ctx.enter_context(tc.tile_pool(name="hin", bufs=2))
        cpo = ctx.enter_context(tc.tile_pool(name="cin", bufs=2))
        gp = ctx.enter_context(tc.tile_pool(name="gacts", bufs=2))
        lp = ctx.enter_context(tc.tile_pool(name="lstm", bufs=3))
        zp = ctx.enter_context(tc.tile_pool(name="zbuf", bufs=2))
        ztp = ctx.enter_context(tc.tile_pool(name="ztok", bufs=1))
        pp = ctx.enter_context(tc.tile_pool(name="pas", bufs=3))
        pzp = ctx.enter_context(tc.tile_pool(name="paz", bufs=3))
        mip = ctx.enter_context(tc.tile_pool(name="mapi", bufs=3))
        a1p = ctx.enter_context(tc.tile_pool(name="a1b", bufs=3))
        a2p = ctx.enter_context(tc.tile_pool(name="a2b", bufs=2))
        avp = ctx.enter_context(tc.tile_pool(name="avb", bufs=2))
        ps_g = ctx.enter_context(tc.tile_pool(name="ps_g", bufs=2, space="PSUM"))
        ps_z = ctx.enter_context(tc.tile_pool(name="ps_z", bufs=2, space="PSUM"))
        ps_a1 = ctx.enter_context(tc.tile_pool(name="ps_a1", bufs=2, space="PSUM"))
        ps_a23 = ctx.enter_context(tc.tile_pool(name="ps_a23", bufs=2, space="PSUM"))
        if True:
            # ---- constants ------------------------------------------
            wih = cp.tile([128, 8, 4 * H], f16)
            nc.sync.dma_start(wih[:], wih_d.ap().rearrange("(k p) m -> p k m", p=128))
            whh = cp.tile([128, 2, 4 * H], f16)
            nc.sync.dma_start(whh[:], whh_d.ap().rearrange("(k p) m -> p k m", p=128))
            w1 = cp.tile([128, 2, 128], f16)
            nc.sync.dma_start(w1[:], w1_d.ap().rearrange("(k p) m -> p k m", p=128))
            w2 = cp.tile([128, 128], f16)
            nc.sync.dma_start(w2[:], w2_d.ap())
            wa1s = cp.tile([ASZ, 64], f16)
            nc.sync.dma_start(wa1s[:], wa1s_d.ap())
            wa1z = cp.tile([128, 64], f16)
            nc.sync.dma_start(wa1z[:], wa1z_d.ap())
            wa2 = cp.tile([128, 32], f16)
            nc.sync.dma_start(wa2[:], wa2_d.ap())
            wa3 = cp.tile([128, 32], f16)
            nc.sync.dma_start(wa3[:], wa3_d.ap())
            bsum = cp.tile([128, 8], f32)
            nc.sync.dma_start(bsum[:], bsum_d.ap())
            b1 = cp.tile([128, 1], f32)
            nc.sync.dma_start(b1[:], b1_d.ap())
            b2bc = cp.tile([128, 128], f32)
            nc.sync.dma_start(b2bc[:], b2bc_d.ap())
            ba1 = cp.tile([128, 1], f32)
            nc.sync.dma_start(ba1[:], ba1_d.ap())
            ba2 = cp.tile([128, 1], f32)
            nc.sync.dma_start(ba2[:], ba2_d.ap())
            ba3 = cp.tile([128, 1], f32)
            nc.sync.dma_start(ba3[:], ba3_d.ap())

            z2tok = ztp.tile([128, 4 * NJ, 128], f16)  # token-major z2

            xT_r = xT_d.ap().rearrange("(k p) n -> p k n", p=128)
            h0T_r = h0T_d.ap().rearrange("(k p) n -> p k n", p=128)
            c0T_r = c0T_d.ap().rearrange("(k p) n -> p k n", p=128)

            # ---- LSTM + state MLP, 512 states at a time --------------
            for j in range(NJ):
                cs = slice(j * 512, (j + 1) * 512)
                xj = xp.tile([128, 8, 512], f16)
                nc.sync.dma_start(xj[:], xT_r[:, :, cs])
                hj = hp.tile([128, 2, 512], f16)
                nc.sync.dma_start(hj[:], h0T_r[:, :, cs])
                cj = cpo.tile([128, 2, 512], f32)
                nc.sync.dma_start(cj[:], c0T_r[:, :, cs])

                ga = gp.tile([128, 8, 512], f32)
                # order: sigmoid chunks (i,f,o) then tanh chunks (g)
                for m in (0, 1, 2, 3, 6, 7, 4, 5):
                    pg = ps_g.tile([128, 512], f32, tag="pg")
                    for k in range(8):
                        nc.tensor.matmul(pg[:], wih[:, k, m * 128:(m + 1) * 128],
                                         xj[:, k, :], start=(k == 0), stop=False)
                    for q in range(2):
                        nc.tensor.matmul(pg[:], whh[:, q, m * 128:(m + 1) * 128],
                                         hj[:, q, :], start=False, stop=(q == 1))
                    fn = FT.Tanh if m in (4, 5) else FT.Sigmoid
                    nc.scalar.activation(ga[:, m, :], pg[:], fn,
                                         bias=bsum[:, m:m + 1])

                h1f16 = lp.tile([128, 2, 512], f16, tag="h1f16")
                for p in range(2):
                    t1 = lp.tile([128, 512], f32, tag="t1")
                    nc.vector.tensor_mul(t1[:], ga[:, 2 + p, :], cj[:, p, :])
                    t2 = lp.tile([128, 512], f32, tag="t2")
                    nc.vector.tensor_mul(t2[:], ga[:, 0 + p, :], ga[:, 4 + p, :])
                    c1 = lp.tile([128, 512], f32, tag="c1")
                    nc.vector.tensor_add(c1[:], t1[:], t2[:])
                    nc.sync.dma_start(hcT_d.ap()[H + p * 128:H + (p + 1) * 128, cs],
                                      c1[:])
                    tcn = lp.tile([128, 512], f32, tag="tc")
                    nc.scalar.activation(tcn[:], c1[:], FT.Tanh)
                    h1 = lp.tile([128, 512], f32, tag="h1")
                    nc.vector.tensor_mul(h1[:], ga[:, 6 + p, :], tcn[:])
                    nc.sync.dma_start(hcT_d.ap()[p * 128:(p + 1) * 128, cs], h1[:])
                    nc.vector.tensor_copy(h1f16[:, p, :], h1[:])

                pz1 = ps_z.tile([128, 512], f32, tag="pz")
                for p in range(2):
                    nc.tensor.matmul(pz1[:], w1[:, p, :], h1f16[:, p, :],
                                     start=(p == 0), stop=(p == 1))
                z1 = zp.tile([128, 512], f16, tag="z1")
                nc.scalar.activation(z1[:], pz1[:], FT.Relu, bias=b1[:])
                for t in range(4):
                    pz2 = ps_z.tile([128, 128], f32, tag="pz")
                    nc.tensor.matmul(pz2[:], z1[:, t * 128:(t + 1) * 128], w2[:])
                    zb = zp.tile([128, 128], f32, tag="zb")
                    nc.vector.tensor_add(zb[:], pz2[:], b2bc[:])
                    nc.vector.tensor_scalar_max(z2tok[:, j * 4 + t, :], zb[:], 0.0)

            # ---- action groups: gather + 3-layer MLP ----------------
            for g in range(NG if mode != "lstm" else 0):
                asl = slice(g * CG, (g + 1) * CG)
                mpt = mip.tile([128, CG // 16], i16)
                nc.sync.dma_start(mpt[:], mp_d.ap()[:, g * (CG // 16):(g + 1) * (CG // 16)])
                pas = pp.tile([ASZ, CG], f16)
                nc.sync.dma_start(pas[:], pasT_d.ap()[:, asl])
                paz = pzp.tile([128, 1, CG], f16)
                if mode == "nogather":
                    nc.vector.memset(paz[:], 0.0)
                else:
                    nc.gpsimd.dma_gather(
                        paz[:], z2tok[:], mpt[:], CG, CG, 128,
                        transpose=True, sbuf_tokens_per_rank=128,
                        sbuf_free_dim_per_rank=256, single_packet=False)

                a1t = []
                for pair in range(2):
                    pa1 = ps_a1.tile([128, 512], f32, tag="pa1")
                    for half in range(2):
                        c = pair * 2 + half
                        o = pa1[half * 64:(half + 1) * 64, :]
                        nc.tensor.matmul(o, wa1s[:], pas[:, c * 512:(c + 1) * 512],
                                         start=True, stop=False)
                        nc.tensor.matmul(o, wa1z[:], paz[:, 0, c * 512:(c + 1) * 512],
                                         start=False, stop=True)
                    a1 = a1p.tile([128, 512], f16, tag="a1")
                    if pair == 0:
                        nc.scalar.activation(a1[:], pa1[:], FT.Relu, bias=ba1[:])
                    else:
                        nc.vector.tensor_scalar(a1[:], pa1[:], ba1[:], 0.0,
                                                op0=OP.add, op1=OP.max)
                    a1t.append(a1)

                pa2 = ps_a23.tile([128, 512], f32, tag="pa23")
                for c in range(4):
                    rhs = a1t[c // 2][(c % 2) * 64:(c % 2 + 1) * 64, :]
                    lhs = wa2[(c % 2) * 64:(c % 2 + 1) * 64, :]
                    nc.tensor.matmul(pa2[c * 32:(c + 1) * 32, :], lhs, rhs,
                                     tile_position=((c % 2) * 64, c * 32))
                a2s = a2p.tile([128, 512], f16, tag="a2s")
                nc.vector.tensor_scalar(a2s[:], pa2[:], ba2[:], 0.0,
                                        op0=OP.add, op1=OP.max)

                pa3 = ps_a23.tile([128, 512], f32, tag="pa23")
                for c in range(4):
                    nc.tensor.matmul(pa3[c * 32:(c + 1) * 32, :],
                                     wa3[c * 32:c * 32 + 4, 0:32],
                                     a2s[c * 32:c * 32 + 4, :],
                                     tile_position=(c * 32, c * 32))
                av4 = avp.tile([128, 512], f32, tag="av4")
                nc.scalar.activation(av4[:], pa3[:], FT.Identity, bias=ba3[:])
                av4_strided = av4[:, :].rearrange("(a b) n -> a b n", b=32)[:, 0, :]
                nc.sync.dma_start(av4_d.ap()[:, g * 512:(g + 1) * 512], av4_strided)

    nc.compile()
    return nc


def _get_program(NA, mode="full"):
    key = (NA, mode)
    if key not in _PROGRAM_CACHE:
        _PROGRAM_CACHE[key] = _build_program(NA, mode)
    return _PROGRAM_CACHE[key]


def _host_post(results, NA, perms):
    a = np.zeros((A, 1), np.float32)
    hc = np.zeros((B, 2 * H), np.float32)
    for d in range(NCORES):
        r = results[d]
        hc[d * BL:(d + 1) * BL] = r["hcT"].T
        G = NA // CG
        a_core = r["av4"].reshape(4, G, 512).transpose(1, 0, 2).reshape(-1)
        a[perms[d], 0] = a_core[:perms[d].size]
    return a, hc


def kernel(**inputs):
    from concourse.bass_utils import run_bass_kernel_spmd
    in_maps, NA, perms = _host_prep(**{k: np.asarray(v) for k, v in inputs.items()})
    nc = _get_program(NA)
    res = run_bass_kernel_spmd(nc, in_maps, core_ids=list(range(NCORES)))
    return _host_post(res.results, NA, perms)
